# revision 3
# baseline (speedup 1.0000x reference)
"""Cached-attention kernel for Trainium2 (8 NeuronCores, Bass/Tile).

Problem: B=4, L=2048 new tokens, S=2048 cached tokens, D=2048.
  Q = x @ Wq.T ; K = x @ Wk.T ; V = x @ Wv.T
  K_cal = concat(K, cache_k) ; V_cal = concat(V, cache_v)
  out = softmax(Q @ K_cal.T / sqrt(D)) @ V_cal

Sharding: 8 cores = (batch b in 0..3) x (key-half h in 0..1). Each core
handles ALL queries of its batch against HALF the keys (1024 cached +
1024 new); per-core K/V projections cover only its half of the new
tokens.  Softmax is computed flash-style without max subtraction
(scores are O(6) here, exp is safe in fp32): each core returns the
un-normalized numerator sum_p(s) * V (transposed, [D, L]) and the
denominator sum_p(s) [L]; the host combines the two halves exactly.

All matmuls run in float32r (TF32-like: ~1.5e-4 rms rel error, 4x the
throughput of fp32 on the PE array).  PSUM accumulation is fp32.

Layouts are arranged so no on-device transposes are needed: the host
feeds x^T, W^T and cache_k^T; the kernel emits the numerator
transposed and the host transposes back (free on host).
"""

import numpy as np

import concourse.bass as bass
import concourse.tile as tile
from concourse import bacc, mybir
from concourse import bass2jax

F32 = mybir.dt.float32
F32R = mybir.dt.float32r

D = 2048          # model dim (= projection output dim)
L = 2048          # new tokens (queries)
HALF = 1024       # per-core share of new tokens / cached tokens
NT = D // 128     # 16 tiles of 128 along D/E/L
SCALE = 1.0 / float(np.sqrt(D))
N_CORES = 8

_NC_CACHE = {}


def build_program():
    if "nc" in _NC_CACHE:
        return _NC_CACHE["nc"]
    nc = bacc.Bacc(None, target_bir_lowering=False, debug=False)
    xT = nc.dram_tensor("xT", [D, L], F32R, kind="ExternalInput")
    xkvT = nc.dram_tensor("xkvT", [D, HALF], F32R, kind="ExternalInput")
    wqT = nc.dram_tensor("wqT", [D, D], F32R, kind="ExternalInput")
    wkT = nc.dram_tensor("wkT", [D, D], F32R, kind="ExternalInput")
    wvT = nc.dram_tensor("wvT", [D, D], F32R, kind="ExternalInput")
    kcT = nc.dram_tensor("kcT", [D, HALF], F32R, kind="ExternalInput")
    vc = nc.dram_tensor("vc", [HALF, D], F32R, kind="ExternalInput")
    outT = nc.dram_tensor("outT", [D, L], F32, kind="ExternalOutput")
    den = nc.dram_tensor("den", [1, L], F32, kind="ExternalOutput")

    xT_r = xT.rearrange("(t p) l -> p t l", p=128)
    xkvT_r = xkvT.rearrange("(t p) s -> p t s", p=128)
    wqT_r = wqT.rearrange("(t p) e -> p t e", p=128)
    wkT_r = wkT.rearrange("(t p) e -> p t e", p=128)
    wvT_r = wvT.rearrange("(t p) e -> p t e", p=128)
    kcT_r = kcT.rearrange("(t p) s -> p t s", p=128)
    vc_r = vc.rearrange("(t p) d -> p t d", p=128)

    with tile.TileContext(nc) as tc:
        with tc.tile_pool(name="dram", bufs=1, space="DRAM") as dpool:
            qt_d = dpool.tile([D, L], F32R, tag="qt")
            kt_d = dpool.tile([D, HALF], F32R, tag="kt")
            v_d = dpool.tile([HALF, D], F32R, tag="vd")
            qt_dr = qt_d[:].rearrange("(t p) l -> p t l", p=128)
            kt_dr = kt_d[:].rearrange("(t p) s -> p t s", p=128)
            v_dr = v_d[:].rearrange("(t p) d -> p t d", p=128)

            # ---------- Phase Q: QT[e, l] = Wq @ x^T ----------
            with (
                tc.tile_pool(name="xt", bufs=1) as xpool,
                tc.tile_pool(name="wq", bufs=3) as wpool,
                tc.tile_pool(name="qo", bufs=4) as opool,
                tc.tile_pool(name="psQ", bufs=4, space="PSUM") as pspool,
            ):
                # load x^T in 4 column chunks so the first matmuls start early
                xt_c = []
                for lc in range(4):
                    t = xpool.tile([128, NT, 512], F32R, tag=f"xt{lc}")
                    nc.sync.dma_start(t[:], xT_r[:, :, lc * 512:(lc + 1) * 512])
                    xt_c.append(t)
                for et in range(NT):
                    w_sb = wpool.tile([128, NT, 128], F32R, tag="w")
                    nc.sync.dma_start(w_sb[:], wqT_r[:, :, et * 128:(et + 1) * 128])
                    for lc in range(4):
                        ps = pspool.tile([128, 512], F32, tag="ps")
                        for dt in range(NT):
                            nc.tensor.matmul(
                                ps[:],
                                w_sb[:, dt, :],
                                xt_c[lc][:, dt, :],
                                start=(dt == 0),
                                stop=(dt == NT - 1),
                            )
                        o_sb = opool.tile([128, 512], F32R, tag="o")
                        nc.vector.tensor_copy(o_sb[:], ps[:])
                        nc.sync.dma_start(
                            qt_d[et * 128:(et + 1) * 128, lc * 512:(lc + 1) * 512],
                            o_sb[:],
                        )

            # ---------- Phase K/V: KT[e, s_new], V[s_new, d] ----------
            with (
                tc.tile_pool(name="xkv", bufs=1) as xkpool,
                tc.tile_pool(name="wk2", bufs=3) as wkpool,
                tc.tile_pool(name="wv2", bufs=2) as wvpool,
                tc.tile_pool(name="kvo", bufs=4) as kvopool,
                tc.tile_pool(name="psKV", bufs=4, space="PSUM") as pskv,
            ):
                xkv_c = []
                for sc in range(2):
                    t = xkpool.tile([128, NT, 512], F32R, tag=f"xkv{sc}")
                    nc.sync.dma_start(t[:], xkvT_r[:, :, sc * 512:(sc + 1) * 512])
                    xkv_c.append(t)
                # KT[e, s] = Wk @ xkv^T
                for et in range(NT):
                    w_sb = wkpool.tile([128, NT, 128], F32R, tag="wk")
                    nc.sync.dma_start(w_sb[:], wkT_r[:, :, et * 128:(et + 1) * 128])
                    for sc in range(2):
                        ps = pskv.tile([128, 512], F32, tag="ps")
                        for dt in range(NT):
                            nc.tensor.matmul(
                                ps[:],
                                w_sb[:, dt, :],
                                xkv_c[sc][:, dt, :],
                                start=(dt == 0),
                                stop=(dt == NT - 1),
                            )
                        o_sb = kvopool.tile([128, 512], F32R, tag="o")
                        nc.vector.tensor_copy(o_sb[:], ps[:])
                        nc.sync.dma_start(
                            kt_d[et * 128:(et + 1) * 128, sc * 512:(sc + 1) * 512],
                            o_sb[:],
                        )
                # V[s, d] = x_kv @ Wv^T  (natural layout; lhsT = xkv^T tiles)
                for dc in range(4):
                    wv_sb = wvpool.tile([128, NT, 512], F32R, tag="wv")
                    nc.sync.dma_start(wv_sb[:], wvT_r[:, :, dc * 512:(dc + 1) * 512])
                    for st in range(8):
                        sc, so = divmod(st, 4)
                        ps = pskv.tile([128, 512], F32, tag="ps")
                        for dt in range(NT):
                            nc.tensor.matmul(
                                ps[:],
                                xkv_c[sc][:, dt, so * 128:(so + 1) * 128],
                                wv_sb[:, dt, :],
                                start=(dt == 0),
                                stop=(dt == NT - 1),
                            )
                        o_sb = kvopool.tile([128, 512], F32R, tag="o")
                        nc.vector.tensor_copy(o_sb[:], ps[:])
                        nc.sync.dma_start(
                            v_d[st * 128:(st + 1) * 128, dc * 512:(dc + 1) * 512],
                            o_sb[:],
                        )

            # ---------- Phase A: attention ----------
            # local key axis: s-tiles 0..7 = cached half, 8..15 = new half
            with (
                tc.tile_pool(name="qt2", bufs=1) as qpool,
                tc.tile_pool(name="pT", bufs=1) as ppool,
                tc.tile_pool(name="kt2", bufs=3) as kpool,
                tc.tile_pool(name="v2", bufs=3) as vpool,
                tc.tile_pool(name="oA", bufs=4) as oApool,
                tc.tile_pool(name="cst", bufs=1) as cpool,
                tc.tile_pool(name="psS", bufs=3, space="PSUM") as psS,
                tc.tile_pool(name="psO", bufs=3, space="PSUM") as psO,
                tc.tile_pool(name="psD", bufs=2, space="PSUM") as psD,
            ):
                ones_f = cpool.tile([128, 1], F32, tag="ones_f")
                nc.gpsimd.memset(ones_f[:], 1.0)
                ones = cpool.tile([128, 1], F32R, tag="ones")
                nc.vector.tensor_copy(ones[:], ones_f[:])

                for lc2 in range(2):
                    lo = lc2 * HALF
                    qt_sb = qpool.tile([128, NT, HALF], F32R, tag="qt")
                    nc.sync.dma_start(qt_sb[:], qt_dr[:, :, lo:lo + HALF])
                    pT = ppool.tile([128, NT, HALF], F32R, tag="pT")

                    # scores^T [s, l] and p = exp(scale * s)
                    for st in range(NT):
                        kt_sb = kpool.tile([128, NT, 128], F32R, tag="kt")
                        if st < 8:
                            src = kcT_r[:, :, st * 128:(st + 1) * 128]
                        else:
                            src = kt_dr[:, :, (st - 8) * 128:(st - 7) * 128]
                        nc.sync.dma_start(kt_sb[:], src)
                        for ls in range(2):
                            ps = psS.tile([128, 512], F32, tag="psS")
                            for et in range(NT):
                                nc.tensor.matmul(
                                    ps[:],
                                    kt_sb[:, et, :],
                                    qt_sb[:, et, ls * 512:(ls + 1) * 512],
                                    start=(et == 0),
                                    stop=(et == NT - 1),
                                )
                            nc.scalar.activation(
                                pT[:, st, ls * 512:(ls + 1) * 512],
                                ps[:],
                                mybir.ActivationFunctionType.Exp,
                                scale=SCALE,
                            )

                    # numerator^T [d, l] = V^T-tiles contracted with p
                    for dt in range(NT):
                        v_sb = vpool.tile([128, NT, 128], F32R, tag="v")
                        nc.sync.dma_start(
                            v_sb[:, 0:8, :], vc_r[:, :, dt * 128:(dt + 1) * 128]
                        )
                        nc.sync.dma_start(
                            v_sb[:, 8:NT, :], v_dr[:, :, dt * 128:(dt + 1) * 128]
                        )
                        for ls in range(2):
                            ps_o = psO.tile([128, 512], F32, tag="psO")
                            for st in range(NT):
                                nc.tensor.matmul(
                                    ps_o[:],
                                    v_sb[:, st, :],
                                    pT[:, st, ls * 512:(ls + 1) * 512],
                                    start=(st == 0),
                                    stop=(st == NT - 1),
                                )
                            o_sb = oApool.tile([128, 512], F32, tag="o")
                            nc.vector.tensor_copy(o_sb[:], ps_o[:])
                            nc.sync.dma_start(
                                outT[dt * 128:(dt + 1) * 128,
                                     lo + ls * 512:lo + (ls + 1) * 512],
                                o_sb[:],
                            )

                    # denominator [1, l] = ones^T @ p
                    for ls in range(2):
                        ps_d = psD.tile([1, 512], F32, tag="psD")
                        for st in range(NT):
                            nc.tensor.matmul(
                                ps_d[:],
                                ones[:],
                                pT[:, st, ls * 512:(ls + 1) * 512],
                                start=(st == 0),
                                stop=(st == NT - 1),
                            )
                        d_sb = oApool.tile([1, 512], F32, tag="d")
                        nc.vector.tensor_copy(d_sb[:], ps_d[:])
                        nc.sync.dma_start(
                            den[0:1, lo + ls * 512:lo + (ls + 1) * 512], d_sb[:]
                        )
    nc.compile()
    _NC_CACHE["nc"] = nc
    return nc


def make_in_maps(x, cache_k, cache_v, Wq, Wk, Wv):
    """Per-core input maps for the SPMD launch. Core c = (b, h) with
    b = c // 2, h = c % 2."""
    f32 = np.float32
    wqT = np.ascontiguousarray(np.asarray(Wq, f32).T)
    wkT = np.ascontiguousarray(np.asarray(Wk, f32).T)
    wvT = np.ascontiguousarray(np.asarray(Wv, f32).T)
    in_maps = []
    for c in range(N_CORES):
        b, h = divmod(c, 2)
        xb = np.asarray(x[b], f32)
        sl = slice(h * HALF, (h + 1) * HALF)
        in_maps.append({
            "xT": np.ascontiguousarray(xb.T),
            "xkvT": np.ascontiguousarray(xb[sl].T),
            "wqT": wqT,
            "wkT": wkT,
            "wvT": wvT,
            "kcT": np.ascontiguousarray(np.asarray(cache_k[b, sl], f32).T),
            "vc": np.ascontiguousarray(np.asarray(cache_v[b, sl], f32)),
        })
    return in_maps


def combine(results):
    """Host combine: out[b] = ((numT_h0 + numT_h1) / (den_h0 + den_h1)).T"""
    B = N_CORES // 2
    out = np.empty((B, L, D), np.float32)
    for b in range(B):
        r0, r1 = results[2 * b], results[2 * b + 1]
        num = r0["outT"].astype(np.float64) + r1["outT"].astype(np.float64)
        dent = r0["den"][0].astype(np.float64) + r1["den"][0].astype(np.float64)
        out[b] = (num / dent[None, :]).T.astype(np.float32)
    return out


def kernel(x, cache_k, cache_v, Wq, Wk, Wv):
    nc = build_program()
    in_maps = make_in_maps(x, cache_k, cache_v, Wq, Wk, Wv)
    results = bass2jax.run_bass_via_pjrt(nc, in_maps, n_cores=N_CORES)
    return combine(results)


# revision 5
# speedup vs baseline: 55.4970x; 55.4970x over previous
"""Cached-attention kernel for Trainium2 (8 NeuronCores, Bass/Tile).

Problem: B=4, L=2048 new tokens, S=2048 cached tokens, D=2048.
  Q = x @ Wq.T ; K = x @ Wk.T ; V = x @ Wv.T
  K_cal = concat(K, cache_k) ; V_cal = concat(V, cache_v)
  out = softmax(Q @ K_cal.T / sqrt(D)) @ V_cal

Sharding: 8 cores = (batch b in 0..3) x (key-half h in 0..1). Each core
handles ALL queries of its batch against HALF the keys (1024 cached +
1024 new); per-core K/V projections cover only its half of the new
tokens.  Softmax is computed flash-style without max subtraction
(scores are O(6) here, exp is safe in fp32): each core returns the
un-normalized numerator sum_p(s) * V (transposed, [D, L]) and the
denominator sum_p(s) [L]; the host combines the two halves exactly.

All matmuls run in float32r (TF32-like: ~1.5e-4 rms rel error, 4x the
throughput of fp32 on the PE array).  PSUM accumulation is fp32.

Layouts are arranged so no on-device transposes are needed: the host
feeds x^T, W^T and cache_k^T; the kernel emits the numerator
transposed and the host transposes back (free on host).
"""

import numpy as np

import concourse.bass as bass
import concourse.tile as tile
from concourse import bacc, mybir
from concourse import bass2jax

F32 = mybir.dt.float32
F32R = mybir.dt.float32r

D = 2048          # model dim (= projection output dim)
L = 2048          # new tokens (queries)
HALF = 1024       # per-core share of new tokens / cached tokens
NT = D // 128     # 16 tiles of 128 along D/E/L
SCALE = 1.0 / float(np.sqrt(D))
N_CORES = 8

_NC_CACHE = {}


def build_program(reps=1):
    key = ("nc", reps)
    if key in _NC_CACHE:
        return _NC_CACHE[key]
    nc = bacc.Bacc(None, target_bir_lowering=False, debug=False)
    xT = nc.dram_tensor("xT", [D, L], F32R, kind="ExternalInput")
    xkvT = nc.dram_tensor("xkvT", [D, HALF], F32R, kind="ExternalInput")
    wqT = nc.dram_tensor("wqT", [D, D], F32R, kind="ExternalInput")
    wkT = nc.dram_tensor("wkT", [D, D], F32R, kind="ExternalInput")
    wvT = nc.dram_tensor("wvT", [D, D], F32R, kind="ExternalInput")
    kcT = nc.dram_tensor("kcT", [D, HALF], F32R, kind="ExternalInput")
    vc = nc.dram_tensor("vc", [HALF, D], F32R, kind="ExternalInput")
    outT = nc.dram_tensor("outT", [D, L], F32, kind="ExternalOutput")
    den = nc.dram_tensor("den", [1, L], F32, kind="ExternalOutput")

    from contextlib import ExitStack
    with tile.TileContext(nc) as tc:
        with ExitStack() as _rep_stack:
            if reps > 1:
                _rep_stack.enter_context(
                    tc.For_i(0, reps, 1, hint_engines=tuple(mybir.EngineType))
                )
            _emit_body(nc, tc, xT, xkvT, wqT, wkT, wvT, kcT, vc, outT, den)
    nc.compile()
    _NC_CACHE[key] = nc
    return nc


def _emit_body(nc, tc, xT, xkvT, wqT, wkT, wvT, kcT, vc, outT, den):

    xT_r = xT.rearrange("(t p) l -> p t l", p=128)
    xkvT_r = xkvT.rearrange("(t p) s -> p t s", p=128)
    wqT_r = wqT.rearrange("(t p) e -> p t e", p=128)
    wkT_r = wkT.rearrange("(t p) e -> p t e", p=128)
    wvT_r = wvT.rearrange("(t p) e -> p t e", p=128)
    kcT_r = kcT.rearrange("(t p) s -> p t s", p=128)
    vc_r = vc.rearrange("(t p) d -> p t d", p=128)

    if True:
        with tc.tile_pool(name="dram", bufs=1, space="DRAM") as dpool:
            qt_d = dpool.tile([D, L], F32R, tag="qt")
            kt_d = dpool.tile([D, HALF], F32R, tag="kt")
            v_d = dpool.tile([HALF, D], F32R, tag="vd")
            qt_dr = qt_d[:].rearrange("(t p) l -> p t l", p=128)
            kt_dr = kt_d[:].rearrange("(t p) s -> p t s", p=128)
            v_dr = v_d[:].rearrange("(t p) d -> p t d", p=128)

            # ---------- Phase Q: QT[e, l] = Wq @ x^T ----------
            with (
                tc.tile_pool(name="xt", bufs=1) as xpool,
                tc.tile_pool(name="wq", bufs=3) as wpool,
                tc.tile_pool(name="qo", bufs=4) as opool,
                tc.tile_pool(name="psQ", bufs=4, space="PSUM") as pspool,
            ):
                # load x^T in 4 column chunks so the first matmuls start early
                xt_c = []
                for lc in range(4):
                    t = xpool.tile([128, NT, 512], F32R, tag=f"xt{lc}")
                    nc.sync.dma_start(t[:], xT_r[:, :, lc * 512:(lc + 1) * 512])
                    xt_c.append(t)
                for et in range(NT):
                    w_sb = wpool.tile([128, NT, 128], F32R, tag="w")
                    nc.sync.dma_start(w_sb[:], wqT_r[:, :, et * 128:(et + 1) * 128])
                    for lc in range(4):
                        ps = pspool.tile([128, 512], F32, tag="ps")
                        for dt in range(NT):
                            nc.tensor.matmul(
                                ps[:],
                                w_sb[:, dt, :],
                                xt_c[lc][:, dt, :],
                                start=(dt == 0),
                                stop=(dt == NT - 1),
                            )
                        o_sb = opool.tile([128, 512], F32R, tag="o")
                        nc.vector.tensor_copy(o_sb[:], ps[:])
                        nc.sync.dma_start(
                            qt_d[et * 128:(et + 1) * 128, lc * 512:(lc + 1) * 512],
                            o_sb[:],
                        )

            # ---------- Phase K/V: KT[e, s_new], V[s_new, d] ----------
            with (
                tc.tile_pool(name="xkv", bufs=1) as xkpool,
                tc.tile_pool(name="wk2", bufs=3) as wkpool,
                tc.tile_pool(name="wv2", bufs=2) as wvpool,
                tc.tile_pool(name="kvo", bufs=4) as kvopool,
                tc.tile_pool(name="psKV", bufs=4, space="PSUM") as pskv,
            ):
                xkv_c = []
                for sc in range(2):
                    t = xkpool.tile([128, NT, 512], F32R, tag=f"xkv{sc}")
                    nc.sync.dma_start(t[:], xkvT_r[:, :, sc * 512:(sc + 1) * 512])
                    xkv_c.append(t)
                # KT[e, s] = Wk @ xkv^T
                for et in range(NT):
                    w_sb = wkpool.tile([128, NT, 128], F32R, tag="wk")
                    nc.sync.dma_start(w_sb[:], wkT_r[:, :, et * 128:(et + 1) * 128])
                    for sc in range(2):
                        ps = pskv.tile([128, 512], F32, tag="ps")
                        for dt in range(NT):
                            nc.tensor.matmul(
                                ps[:],
                                w_sb[:, dt, :],
                                xkv_c[sc][:, dt, :],
                                start=(dt == 0),
                                stop=(dt == NT - 1),
                            )
                        o_sb = kvopool.tile([128, 512], F32R, tag="o")
                        nc.vector.tensor_copy(o_sb[:], ps[:])
                        nc.sync.dma_start(
                            kt_d[et * 128:(et + 1) * 128, sc * 512:(sc + 1) * 512],
                            o_sb[:],
                        )
                # V[s, d] = x_kv @ Wv^T  (natural layout; lhsT = xkv^T tiles)
                for dc in range(4):
                    wv_sb = wvpool.tile([128, NT, 512], F32R, tag="wv")
                    nc.sync.dma_start(wv_sb[:], wvT_r[:, :, dc * 512:(dc + 1) * 512])
                    for st in range(8):
                        sc, so = divmod(st, 4)
                        ps = pskv.tile([128, 512], F32, tag="ps")
                        for dt in range(NT):
                            nc.tensor.matmul(
                                ps[:],
                                xkv_c[sc][:, dt, so * 128:(so + 1) * 128],
                                wv_sb[:, dt, :],
                                start=(dt == 0),
                                stop=(dt == NT - 1),
                            )
                        o_sb = kvopool.tile([128, 512], F32R, tag="o")
                        nc.vector.tensor_copy(o_sb[:], ps[:])
                        nc.sync.dma_start(
                            v_d[st * 128:(st + 1) * 128, dc * 512:(dc + 1) * 512],
                            o_sb[:],
                        )

            # ---------- Phase A: attention ----------
            # local key axis: s-tiles 0..7 = cached half, 8..15 = new half
            with (
                tc.tile_pool(name="qt2", bufs=1) as qpool,
                tc.tile_pool(name="pT", bufs=1) as ppool,
                tc.tile_pool(name="kt2", bufs=3) as kpool,
                tc.tile_pool(name="v2", bufs=3) as vpool,
                tc.tile_pool(name="oA", bufs=4) as oApool,
                tc.tile_pool(name="cst", bufs=1) as cpool,
                tc.tile_pool(name="psS", bufs=3, space="PSUM") as psS,
                tc.tile_pool(name="psO", bufs=3, space="PSUM") as psO,
                tc.tile_pool(name="psD", bufs=2, space="PSUM") as psD,
            ):
                ones_f = cpool.tile([128, 1], F32, tag="ones_f")
                nc.gpsimd.memset(ones_f[:], 1.0)
                ones = cpool.tile([128, 1], F32R, tag="ones")
                nc.vector.tensor_copy(ones[:], ones_f[:])

                for lc2 in range(2):
                    lo = lc2 * HALF
                    qt_sb = qpool.tile([128, NT, HALF], F32R, tag="qt")
                    nc.sync.dma_start(qt_sb[:], qt_dr[:, :, lo:lo + HALF])
                    pT = ppool.tile([128, NT, HALF], F32R, tag="pT")

                    # scores^T [s, l] and p = exp(scale * s)
                    for st in range(NT):
                        kt_sb = kpool.tile([128, NT, 128], F32R, tag="kt")
                        if st < 8:
                            src = kcT_r[:, :, st * 128:(st + 1) * 128]
                        else:
                            src = kt_dr[:, :, (st - 8) * 128:(st - 7) * 128]
                        nc.sync.dma_start(kt_sb[:], src)
                        for ls in range(2):
                            ps = psS.tile([128, 512], F32, tag="psS")
                            for et in range(NT):
                                nc.tensor.matmul(
                                    ps[:],
                                    kt_sb[:, et, :],
                                    qt_sb[:, et, ls * 512:(ls + 1) * 512],
                                    start=(et == 0),
                                    stop=(et == NT - 1),
                                )
                            nc.scalar.activation(
                                pT[:, st, ls * 512:(ls + 1) * 512],
                                ps[:],
                                mybir.ActivationFunctionType.Exp,
                                scale=SCALE,
                            )

                    # numerator^T [d, l] = V^T-tiles contracted with p
                    for dt in range(NT):
                        v_sb = vpool.tile([128, NT, 128], F32R, tag="v")
                        nc.sync.dma_start(
                            v_sb[:, 0:8, :], vc_r[:, :, dt * 128:(dt + 1) * 128]
                        )
                        nc.sync.dma_start(
                            v_sb[:, 8:NT, :], v_dr[:, :, dt * 128:(dt + 1) * 128]
                        )
                        for ls in range(2):
                            ps_o = psO.tile([128, 512], F32, tag="psO")
                            for st in range(NT):
                                nc.tensor.matmul(
                                    ps_o[:],
                                    v_sb[:, st, :],
                                    pT[:, st, ls * 512:(ls + 1) * 512],
                                    start=(st == 0),
                                    stop=(st == NT - 1),
                                )
                            o_sb = oApool.tile([128, 512], F32, tag="o")
                            nc.vector.tensor_copy(o_sb[:], ps_o[:])
                            nc.sync.dma_start(
                                outT[dt * 128:(dt + 1) * 128,
                                     lo + ls * 512:lo + (ls + 1) * 512],
                                o_sb[:],
                            )

                    # denominator [1, l] = ones^T @ p
                    for ls in range(2):
                        ps_d = psD.tile([1, 512], F32, tag="psD")
                        for st in range(NT):
                            nc.tensor.matmul(
                                ps_d[:],
                                ones[:],
                                pT[:, st, ls * 512:(ls + 1) * 512],
                                start=(st == 0),
                                stop=(st == NT - 1),
                            )
                        d_sb = oApool.tile([1, 512], F32, tag="d")
                        nc.vector.tensor_copy(d_sb[:], ps_d[:])
                        nc.sync.dma_start(
                            den[0:1, lo + ls * 512:lo + (ls + 1) * 512], d_sb[:]
                        )


def make_in_maps(x, cache_k, cache_v, Wq, Wk, Wv):
    """Per-core input maps for the SPMD launch. Core c = (b, h) with
    b = c // 2, h = c % 2."""
    f32 = np.float32
    wqT = np.ascontiguousarray(np.asarray(Wq, f32).T)
    wkT = np.ascontiguousarray(np.asarray(Wk, f32).T)
    wvT = np.ascontiguousarray(np.asarray(Wv, f32).T)
    in_maps = []
    for c in range(N_CORES):
        b, h = divmod(c, 2)
        xb = np.asarray(x[b], f32)
        sl = slice(h * HALF, (h + 1) * HALF)
        in_maps.append({
            "xT": np.ascontiguousarray(xb.T),
            "xkvT": np.ascontiguousarray(xb[sl].T),
            "wqT": wqT,
            "wkT": wkT,
            "wvT": wvT,
            "kcT": np.ascontiguousarray(np.asarray(cache_k[b, sl], f32).T),
            "vc": np.ascontiguousarray(np.asarray(cache_v[b, sl], f32)),
        })
    return in_maps


def combine(results):
    """Host combine: out[b] = ((numT_h0 + numT_h1) / (den_h0 + den_h1)).T"""
    B = N_CORES // 2
    out = np.empty((B, L, D), np.float32)
    for b in range(B):
        r0, r1 = results[2 * b], results[2 * b + 1]
        num = r0["outT"].astype(np.float64) + r1["outT"].astype(np.float64)
        dent = r0["den"][0].astype(np.float64) + r1["den"][0].astype(np.float64)
        out[b] = (num / dent[None, :]).T.astype(np.float32)
    return out


def kernel(x, cache_k, cache_v, Wq, Wk, Wv):
    nc = build_program()
    in_maps = make_in_maps(x, cache_k, cache_v, Wq, Wk, Wv)
    results = bass2jax.run_bass_via_pjrt(nc, in_maps, n_cores=N_CORES)
    return combine(results)
